# revision 4
# baseline (speedup 1.0000x reference)
"""Trainium2 Bass kernel for nn_DecoderTransformer segment_reduce problem.

Computes, per batch sample b (one NeuronCore each, 8 cores total):
    sums[s, :]   = sum over (n, k) with indexes[b, n, k] == s of graph_output[b, n, :]
    counts[s]    = multiplicity of s in indexes[b]
    graph_hidden = (sums + 1e-8) / max(counts, 1)
    enc[b]       = concat([graph_hidden, seq_output[b]], axis=-1)   # [2048, 1024]
Returns (enc [8, 2048, 1024] f32, hidden [8, 1024] f32 passthrough).

Device algorithm (per core):
  Host sorts the 2048 (n, k) updates by target s; the sorted stream is cut in
  16 chunks of 128. The source rows of graph_output are fetched in sorted
  order by 4 dma_gather ops (512 rows each; rows land on partitions in chunk
  layout [128, 4, 512]). Per chunk, one is_equal tensor_scalar against an
  iota row builds the selection matrix for the window of output tiles the
  chunk's targets span (union across cores so the program is SPMD-uniform).
  The scatter-add is Sel.T @ rows on the tensor engine in float32r hi/lo
  split form (two 1-cycle/row matmuls, full fp32 accuracy), accumulated in
  PSUM across chunks. The PSUM->SBUF pass fuses (sums + eps) * r with
  r = 1/max(counts, 1) host-precomputed from the index metadata, writing the
  left half of a [128, 1024] tile whose right half receives seq_output, so
  each output tile stores as one contiguous 512 KiB DMA.
"""

import numpy as np

import concourse.bass as bass
import concourse.bacc as bacc
import concourse.tile as tile
from concourse import mybir
from concourse.bass_utils import run_bass_kernel_spmd

B, S, N, K = 8, 2048, 512, 4
DG, DSEQ, H = 512, 512, 1024
P = 128
N_CHUNKS = (N * K) // P  # 16
N_TILES = S // P  # 16
N_GATHERS = 4  # 2048 rows in 4 dma_gather ops (>1024 idxs per op crashes Q7)
CHUNKS_PER_GATHER = N_CHUNKS // N_GATHERS
EPS = 1e-8

# Filled by kernel() on every call; read by test harnesses.
LAST_EXEC_NS = None
LAST_RESULTS = None


def _host_metadata(indexes):
    """Per-core sorted-update metadata + SPMD-uniform chunk->tile windows."""
    per_core = []
    for b in range(B):
        t_flat = np.asarray(indexes[b], dtype=np.int64).reshape(-1)  # (n, k) order
        order = np.argsort(t_flat, kind="stable")
        t_sorted = t_flat[order]
        src = (order // K).astype(np.int16)
        counts = np.bincount(t_flat, minlength=S)
        r = (1.0 / np.maximum(counts, 1)).astype(np.float32)
        per_core.append((t_sorted, src, r))

    # Union coverage: chunk c touches output tiles [lo[c], hi[c]] across cores.
    lo = np.full(N_CHUNKS, N_TILES, np.int64)
    hi = np.full(N_CHUNKS, -1, np.int64)
    for t_sorted, _, _ in per_core:
        tc_lo = t_sorted.reshape(N_CHUNKS, P)[:, 0] // P
        tc_hi = t_sorted.reshape(N_CHUNKS, P)[:, -1] // P
        lo = np.minimum(lo, tc_lo)
        hi = np.maximum(hi, tc_hi)
    lo = lo.astype(int)
    hi = hi.astype(int)

    # per tile: ordered list of (chunk, lhsT slice offset)
    tile_chunks = {tau: [] for tau in range(N_TILES)}
    for c in range(N_CHUNKS):
        for tau in range(lo[c], hi[c] + 1):
            tile_chunks[tau].append((c, P * (tau - lo[c])))

    src_cols = np.zeros((B, P, (N * K) // 16), np.int16)  # wrapped idx layout
    shift_cols = np.zeros((B, P, N_CHUNKS), np.float32)
    r_cols = np.zeros((B, P, N_TILES), np.float32)
    for b in range(B):
        t_sorted, src, r = per_core[b]
        # dma_gather wrapped index layout: idx j of gather g lives at
        # [j % 16, j // 16] within each 16-partition group, replicated x8.
        w = src.reshape(N_CHUNKS * P // 16, 16).T  # [16, 128]
        src_cols[b] = np.tile(w, (8, 1))
        r_cols[b] = r.reshape(N_TILES, P).T
        ts_chunks = t_sorted.reshape(N_CHUNKS, P)
        for c in range(N_CHUNKS):
            shift_cols[b, :, c] = (ts_chunks[c] - P * lo[c]).astype(np.float32)
    return lo, hi, tile_chunks, src_cols, shift_cols, r_cols


def _build_kernel(lo, hi, tile_chunks):
    f32 = mybir.dt.float32
    f32r = mybir.dt.float32r
    spans = [hi[c] - lo[c] + 1 for c in range(N_CHUNKS)]
    wmax = P * max(spans)
    nc = bacc.Bacc("TRN2", target_bir_lowering=False, debug=False)
    seq = nc.dram_tensor("seq", [S, DSEQ], f32, kind="ExternalInput")
    g = nc.dram_tensor("g", [N, DG], f32, kind="ExternalInput")
    srcm = nc.dram_tensor("srcm", [P, (N * K) // 16], mybir.dt.int16,
                          kind="ExternalInput")
    shiftm = nc.dram_tensor("shiftm", [P, N_CHUNKS], f32, kind="ExternalInput")
    rm = nc.dram_tensor("rm", [P, N_TILES], f32, kind="ExternalInput")
    enc = nc.dram_tensor("enc", [S, DG + DSEQ], f32, kind="ExternalOutput")

    idx_cols_per_gather = (P * CHUNKS_PER_GATHER) // 16  # 32

    with tile.TileContext(nc) as tc:
        with (
            tc.tile_pool(name="const", bufs=1) as const,
            tc.tile_pool(name="gath", bufs=N_GATHERS) as gather_pool,
            tc.tile_pool(name="hilo", bufs=N_CHUNKS) as hilo_pool,
            tc.tile_pool(name="eq", bufs=N_CHUNKS) as eq_pool,
            tc.tile_pool(name="out", bufs=6) as out_pool,
            tc.tile_pool(name="psum", bufs=8, space="PSUM") as psum_pool,
        ):
            src_sb = const.tile([P, (N * K) // 16], mybir.dt.int16)
            nc.sync.dma_start(out=src_sb[:], in_=srcm[:, :])
            shift_sb = const.tile([P, N_CHUNKS], f32)
            nc.sync.dma_start(out=shift_sb[:], in_=shiftm[:, :])
            r_sb = const.tile([P, N_TILES], f32)
            nc.sync.dma_start(out=r_sb[:], in_=rm[:, :])

            iota_i = const.tile([P, wmax], mybir.dt.int32)
            nc.gpsimd.iota(iota_i[:], pattern=[[1, wmax]], base=0,
                           channel_multiplier=0)
            iota_f = const.tile([P, wmax], f32)
            nc.vector.tensor_copy(out=iota_f[:], in_=iota_i[:])
            epsr_sb = const.tile([P, N_TILES], f32)
            nc.vector.tensor_scalar(
                out=epsr_sb[:], in0=r_sb[:], scalar1=EPS, scalar2=None,
                op0=mybir.AluOpType.mult,
            )

            # Gather all 2048 source rows in sorted order, 512 per op.
            gathers = []
            for gi in range(N_GATHERS):
                gt = gather_pool.tile([P, CHUNKS_PER_GATHER, DG], f32)
                nc.gpsimd.dma_gather(
                    gt[:], g[:, :],
                    src_sb[:, gi * idx_cols_per_gather:(gi + 1) * idx_cols_per_gather],
                    P * CHUNKS_PER_GATHER, P * CHUNKS_PER_GATHER, DG, elem_step=DG,
                )
                gathers.append(gt)

            # Per chunk: f32r hi/lo split of the gathered rows + selection
            # window matrix.
            ghis, glos, eqs = [], [], []
            for c in range(N_CHUNKS):
                raw = gathers[c // CHUNKS_PER_GATHER][:, c % CHUNKS_PER_GATHER, :]
                ghi = hilo_pool.tile([P, DG], f32r, tag="ghi")
                nc.scalar.copy(out=ghi[:], in_=raw)
                glo = hilo_pool.tile([P, DG], f32r, tag="glo")
                nc.vector.tensor_tensor(
                    out=glo[:], in0=raw, in1=ghi[:].bitcast(f32),
                    op=mybir.AluOpType.subtract,
                )
                eqt = eq_pool.tile([P, P * spans[c]], f32r, tag="eq")
                nc.vector.tensor_scalar(
                    out=eqt[:], in0=iota_f[:, :P * spans[c]],
                    scalar1=shift_sb[:, c:c + 1], scalar2=None,
                    op0=mybir.AluOpType.is_equal,
                )
                ghis.append(ghi)
                glos.append(glo)
                eqs.append(eqt)

            # Per output tile: accumulate hi/lo matmuls, fused epilogue,
            # concat with seq rows, one contiguous store.
            for tau in range(N_TILES):
                ot = out_pool.tile([P, DG + DSEQ], f32)
                nc.sync.dma_start(
                    out=ot[:, DG:], in_=seq[tau * P:(tau + 1) * P, :]
                )
                plist = tile_chunks[tau]
                if plist:
                    ps = psum_pool.tile([P, DG], f32, space="PSUM")
                    nmm = 2 * len(plist)
                    i = 0
                    for c, off in plist:
                        for rhs in (ghis[c], glos[c]):
                            nc.tensor.matmul(
                                out=ps[:], lhsT=eqs[c][:, off:off + P], rhs=rhs[:],
                                start=(i == 0), stop=(i == nmm - 1),
                            )
                            i += 1
                    if tau % 2 == 0:
                        nc.vector.tensor_scalar(
                            out=ot[:, :DG], in0=ps[:], scalar1=EPS,
                            scalar2=r_sb[:, tau:tau + 1],
                            op0=mybir.AluOpType.add, op1=mybir.AluOpType.mult,
                        )
                    else:
                        nc.scalar.activation(
                            out=ot[:, :DG], in_=ps[:],
                            func=mybir.ActivationFunctionType.Identity,
                            bias=epsr_sb[:, tau:tau + 1],
                            scale=r_sb[:, tau:tau + 1],
                        )
                else:
                    nc.vector.memset(ot[:, :DG], EPS)
                nc.sync.dma_start(
                    out=enc[tau * P:(tau + 1) * P, :], in_=ot[:]
                )
    nc.compile()
    return nc


def kernel(seq_output, graph_output, hidden, indexes, _trace=False):
    global LAST_EXEC_NS, LAST_RESULTS
    seq_output = np.ascontiguousarray(np.asarray(seq_output, dtype=np.float32))
    graph_output = np.ascontiguousarray(np.asarray(graph_output, dtype=np.float32))
    hidden_np = np.asarray(hidden)

    lo, hi, tile_chunks, src_cols, shift_cols, r_cols = _host_metadata(indexes)
    nc = _build_kernel(lo, hi, tile_chunks)

    in_maps = [
        {
            "seq": seq_output[b],
            "g": graph_output[b],
            "srcm": np.ascontiguousarray(src_cols[b]),
            "shiftm": np.ascontiguousarray(shift_cols[b]),
            "rm": np.ascontiguousarray(r_cols[b]),
        }
        for b in range(B)
    ]
    res = run_bass_kernel_spmd(nc, in_maps, core_ids=list(range(B)), trace=_trace)
    LAST_EXEC_NS = res.exec_time_ns
    LAST_RESULTS = res
    enc = np.stack([res.results[b]["enc"] for b in range(B)], axis=0)
    hidden_flat = np.ascontiguousarray(hidden_np.reshape(hidden_np.shape[0], -1))
    return enc, hidden_flat


# revision 7
# speedup vs baseline: 1.1448x; 1.1448x over previous
"""Trainium2 Bass kernel for nn_DecoderTransformer segment_reduce problem.

Computes, per batch sample b (one NeuronCore each, 8 cores total):
    sums[s, :]   = sum over (n, k) with indexes[b, n, k] == s of graph_output[b, n, :]
    counts[s]    = multiplicity of s in indexes[b]
    graph_hidden = (sums + 1e-8) / max(counts, 1)
    enc[b]       = concat([graph_hidden, seq_output[b]], axis=-1)   # [2048, 1024]
Returns (enc [8, 2048, 1024] f32, hidden [8, 1024] f32 passthrough).

Device algorithm (per core):
  Host sorts the 2048 (n, k) updates by target s; the sorted stream is cut in
  16 chunks of 128. The source rows of graph_output are fetched in sorted
  order by 4 dma_gather ops (512 rows each; rows land on partitions in chunk
  layout [128, 4, 512]). Each gather batch is split into an exact float32r
  hi/lo pair (ACT copy + DVE subtract). Per chunk, one is_equal tensor_scalar
  against a host-provided iota row builds the selection matrix for the window
  of output tiles the chunk's targets span (window union across cores keeps
  the program SPMD-uniform). The scatter-add is Sel.T @ rows on the tensor
  engine as two float32r matmuls (1 cycle/row each, full fp32 accuracy),
  accumulated in PSUM across chunks. The PSUM->SBUF pass fuses
  (sums + eps) * r with r = 1/max(counts, 1) host-precomputed from the index
  metadata, writing into [128, 4096] super-tiles interleaved with seq_output
  so output stores are 4 contiguous 2 MiB DMAs.
"""

import numpy as np

import concourse.bass as bass
import concourse.bacc as bacc
import concourse.tile as tile
from concourse import mybir
from concourse.bass_utils import run_bass_kernel_spmd

B, S, N, K = 8, 2048, 512, 4
DG, DSEQ, H = 512, 512, 1024
P = 128
N_CHUNKS = (N * K) // P  # 16
N_TILES = S // P  # 16
N_GATHERS = 4  # 2048 rows in 4 dma_gather ops (>1024 idxs per op crashes Q7)
CPG = N_CHUNKS // N_GATHERS  # chunks per gather
TPS = 4  # tiles per output super-tile
EPS = 1e-8

# Filled by kernel() on every call; read by test harnesses.
LAST_EXEC_NS = None
LAST_RESULTS = None


def _host_metadata(indexes):
    """Per-core sorted-update metadata + SPMD-uniform chunk->tile windows."""
    per_core = []
    for b in range(B):
        t_flat = np.asarray(indexes[b], dtype=np.int64).reshape(-1)  # (n, k) order
        order = np.argsort(t_flat, kind="stable")
        t_sorted = t_flat[order]
        src = (order // K).astype(np.int16)
        counts = np.bincount(t_flat, minlength=S)
        r = (1.0 / np.maximum(counts, 1)).astype(np.float32)
        per_core.append((t_sorted, src, r))

    # Union coverage: chunk c touches output tiles [lo[c], hi[c]] across cores.
    lo = np.full(N_CHUNKS, N_TILES, np.int64)
    hi = np.full(N_CHUNKS, -1, np.int64)
    for t_sorted, _, _ in per_core:
        tc_lo = t_sorted.reshape(N_CHUNKS, P)[:, 0] // P
        tc_hi = t_sorted.reshape(N_CHUNKS, P)[:, -1] // P
        lo = np.minimum(lo, tc_lo)
        hi = np.maximum(hi, tc_hi)
    lo = lo.astype(int)
    hi = hi.astype(int)

    # per tile: ordered list of (chunk, lhsT slice offset)
    tile_chunks = {tau: [] for tau in range(N_TILES)}
    for c in range(N_CHUNKS):
        for tau in range(lo[c], hi[c] + 1):
            tile_chunks[tau].append((c, P * (tau - lo[c])))

    src_cols = np.zeros((B, P, (N * K) // 16), np.int16)  # wrapped idx layout
    shift_cols = np.zeros((B, P, N_CHUNKS), np.float32)
    r_cols = np.zeros((B, P, N_TILES), np.float32)
    for b in range(B):
        t_sorted, src, r = per_core[b]
        # dma_gather wrapped index layout: idx j of gather g lives at
        # [j % 16, j // 16] within each 16-partition group, replicated x8.
        w = src.reshape(N_CHUNKS * P // 16, 16).T  # [16, 128]
        src_cols[b] = np.tile(w, (8, 1))
        r_cols[b] = r.reshape(N_TILES, P).T
        ts_chunks = t_sorted.reshape(N_CHUNKS, P)
        for c in range(N_CHUNKS):
            shift_cols[b, :, c] = (ts_chunks[c] - P * lo[c]).astype(np.float32)
    return lo, hi, tile_chunks, src_cols, shift_cols, r_cols


def _build_kernel(lo, hi, tile_chunks):
    f32 = mybir.dt.float32
    f32r = mybir.dt.float32r
    spans = [hi[c] - lo[c] + 1 for c in range(N_CHUNKS)]
    wmax = P * max(spans)
    nc = bacc.Bacc("TRN2", target_bir_lowering=False, debug=False)
    seq = nc.dram_tensor("seq", [S, DSEQ], f32, kind="ExternalInput")
    g = nc.dram_tensor("g", [N, DG], f32, kind="ExternalInput")
    srcm = nc.dram_tensor("srcm", [P, (N * K) // 16], mybir.dt.int16,
                          kind="ExternalInput")
    shiftm = nc.dram_tensor("shiftm", [P, N_CHUNKS], f32, kind="ExternalInput")
    rm = nc.dram_tensor("rm", [P, N_TILES], f32, kind="ExternalInput")
    iotam = nc.dram_tensor("iotam", [P, wmax], f32, kind="ExternalInput")
    enc = nc.dram_tensor("enc", [S, DG + DSEQ], f32, kind="ExternalOutput")

    idx_cols = (P * CPG) // 16  # 32 index columns per gather

    with tile.TileContext(nc) as tc:
        with (
            tc.tile_pool(name="const", bufs=1) as const,
            tc.tile_pool(name="gath", bufs=N_GATHERS) as gather_pool,
            tc.tile_pool(name="hilo", bufs=N_GATHERS) as hilo_pool,
            tc.tile_pool(name="eq", bufs=N_CHUNKS) as eq_pool,
            tc.tile_pool(name="out", bufs=4) as out_pool,
            tc.tile_pool(name="psum", bufs=8, space="PSUM") as psum_pool,
        ):
            # Metadata loads first: the gathers depend only on src_sb.
            src_sb = const.tile([P, (N * K) // 16], mybir.dt.int16)
            nc.sync.dma_start(out=src_sb[:], in_=srcm[:, :])
            shift_sb = const.tile([P, N_CHUNKS], f32)
            nc.sync.dma_start(out=shift_sb[:], in_=shiftm[:, :])
            r_sb = const.tile([P, N_TILES], f32)
            nc.sync.dma_start(out=r_sb[:], in_=rm[:, :])
            iota_f = const.tile([P, wmax], f32)
            nc.sync.dma_start(out=iota_f[:], in_=iotam[:, :])
            epsr_sb = const.tile([P, N_TILES], f32)
            nc.vector.tensor_scalar(
                out=epsr_sb[:], in0=r_sb[:], scalar1=EPS, scalar2=None,
                op0=mybir.AluOpType.mult,
            )

            # Gather all 2048 source rows in sorted order, 512 per op, and
            # split each batch into exact f32r hi/lo parts.
            ghis, glos = [], []
            for gi in range(N_GATHERS):
                gt = gather_pool.tile([P, CPG, DG], f32)
                nc.gpsimd.dma_gather(
                    gt[:], g[:, :],
                    src_sb[:, gi * idx_cols:(gi + 1) * idx_cols],
                    P * CPG, P * CPG, DG, elem_step=DG,
                )
                ghi = hilo_pool.tile([P, CPG, DG], f32r, tag="ghi")
                nc.scalar.copy(out=ghi[:], in_=gt[:])
                glo = hilo_pool.tile([P, CPG, DG], f32r, tag="glo")
                nc.vector.tensor_tensor(
                    out=glo[:], in0=gt[:], in1=ghi[:].bitcast(f32),
                    op=mybir.AluOpType.subtract,
                )
                ghis.append(ghi)
                glos.append(glo)

            # Per chunk: selection window matrix.
            eqs = []
            for c in range(N_CHUNKS):
                eqt = eq_pool.tile([P, P * spans[c]], f32r, tag="eq")
                nc.vector.tensor_scalar(
                    out=eqt[:], in0=iota_f[:, :P * spans[c]],
                    scalar1=shift_sb[:, c:c + 1], scalar2=None,
                    op0=mybir.AluOpType.is_equal,
                )
                eqs.append(eqt)

            # Per output tile: accumulate hi/lo matmuls, fused epilogue into a
            # super-tile interleaved with seq rows; store 2 MiB per 4 tiles.
            for st in range(N_TILES // TPS):
                ot = out_pool.tile([P, TPS, DG + DSEQ], f32)
                # seq rows for the 4 tiles of this super-tile: DRAM rows
                # (tau*128 + p) -> ot[p, tau - st*TPS, 512:1024]
                nc.sync.dma_start(
                    out=ot[:, :, DG:],
                    in_=seq[:, :].rearrange("(t p) d -> p t d", p=P)[
                        :, st * TPS:(st + 1) * TPS, :],
                )
                for ti in range(TPS):
                    tau = st * TPS + ti
                    plist = tile_chunks[tau]
                    oslice = ot[:, ti, :DG]
                    if plist:
                        ps = psum_pool.tile([P, DG], f32, space="PSUM")
                        nmm = 2 * len(plist)
                        i = 0
                        for c, off in plist:
                            gi, ci = c // CPG, c % CPG
                            for rhs in (ghis[gi], glos[gi]):
                                nc.tensor.matmul(
                                    out=ps[:], lhsT=eqs[c][:, off:off + P],
                                    rhs=rhs[:, ci, :],
                                    start=(i == 0), stop=(i == nmm - 1),
                                )
                                i += 1
                        if tau % 2 == 0:
                            nc.vector.tensor_scalar(
                                out=oslice, in0=ps[:], scalar1=EPS,
                                scalar2=r_sb[:, tau:tau + 1],
                                op0=mybir.AluOpType.add, op1=mybir.AluOpType.mult,
                            )
                        else:
                            nc.scalar.activation(
                                out=oslice, in_=ps[:],
                                func=mybir.ActivationFunctionType.Identity,
                                bias=epsr_sb[:, tau:tau + 1],
                                scale=r_sb[:, tau:tau + 1],
                            )
                    else:
                        nc.vector.memset(oslice, EPS)
                nc.sync.dma_start(
                    out=enc[st * TPS * P:(st + 1) * TPS * P, :].rearrange(
                        "(t p) d -> p t d", p=P),
                    in_=ot[:],
                )
    nc.compile()
    return nc


def kernel(seq_output, graph_output, hidden, indexes, _trace=False):
    global LAST_EXEC_NS, LAST_RESULTS
    seq_output = np.ascontiguousarray(np.asarray(seq_output, dtype=np.float32))
    graph_output = np.ascontiguousarray(np.asarray(graph_output, dtype=np.float32))
    hidden_np = np.asarray(hidden)

    lo, hi, tile_chunks, src_cols, shift_cols, r_cols = _host_metadata(indexes)
    nc = _build_kernel(lo, hi, tile_chunks)

    wmax = P * max(hi[c] - lo[c] + 1 for c in range(N_CHUNKS))
    iota_full = np.broadcast_to(
        np.arange(wmax, dtype=np.float32), (P, wmax)
    ).copy()

    in_maps = [
        {
            "seq": seq_output[b],
            "g": graph_output[b],
            "srcm": np.ascontiguousarray(src_cols[b]),
            "shiftm": np.ascontiguousarray(shift_cols[b]),
            "rm": np.ascontiguousarray(r_cols[b]),
            "iotam": iota_full,
        }
        for b in range(B)
    ]
    res = run_bass_kernel_spmd(nc, in_maps, core_ids=list(range(B)), trace=_trace)
    LAST_EXEC_NS = res.exec_time_ns
    LAST_RESULTS = res
    enc = np.stack([res.results[b]["enc"] for b in range(B)], axis=0)
    hidden_flat = np.ascontiguousarray(hidden_np.reshape(hidden_np.shape[0], -1))
    return enc, hidden_flat


# revision 10
# speedup vs baseline: 1.1839x; 1.0341x over previous
"""Trainium2 Bass kernel for nn_DecoderTransformer segment_reduce problem.

Computes, per batch sample b (one NeuronCore each, 8 cores total):
    sums[s, :]   = sum over (n, k) with indexes[b, n, k] == s of graph_output[b, n, :]
    counts[s]    = multiplicity of s in indexes[b]
    graph_hidden = (sums + 1e-8) / max(counts, 1)
    enc[b]       = concat([graph_hidden, seq_output[b]], axis=-1)   # [2048, 1024]
Returns (enc [8, 2048, 1024] f32, hidden [8, 1024] f32 passthrough).

Device algorithm (per core):
  Host sorts the 2048 (n, k) updates by target s; the sorted stream is cut in
  16 chunks of 128. The source rows of graph_output are fetched in sorted
  order by 4 dma_gather ops (512 rows each; rows land on partitions in chunk
  layout [128, 4, 512]). Per chunk the rows are split into an exact float32r
  hi/lo pair (ACT copy + DVE subtract), and one is_equal tensor_scalar
  against a host-provided iota row builds the selection matrix for the window
  of output tiles the chunk's targets span (window union across cores keeps
  the program SPMD-uniform). The scatter-add is Sel.T @ rows on the tensor
  engine as two float32r matmuls, accumulated in PSUM across chunks. The
  PSUM->SBUF pass (ACT) fuses (sums + eps) * r with r = 1/max(counts, 1)
  host-precomputed from the index metadata, writing into [128, 4096]
  super-tiles interleaved with seq_output so output stores are 4 contiguous
  2 MiB DMAs. Instructions are emitted chunk-major in execution order: the
  per-engine queues are strict FIFO, so emission order is schedule order.
"""

import numpy as np

import concourse.bass as bass
import concourse.bacc as bacc
import concourse.tile as tile
from concourse import mybir
from concourse.bass_utils import run_bass_kernel_spmd

B, S, N, K = 8, 2048, 512, 4
DG, DSEQ, H = 512, 512, 1024
P = 128
N_CHUNKS = (N * K) // P  # 16
N_TILES = S // P  # 16
N_GATHERS = 4  # 2048 rows in 4 dma_gather ops (>1024 idxs per op crashes Q7)
CPG = N_CHUNKS // N_GATHERS  # chunks per gather
TPS = 4  # tiles per output super-tile
EPS = 1e-8

# Filled by kernel() on every call; read by test harnesses.
LAST_EXEC_NS = None
LAST_RESULTS = None


def _host_metadata(indexes):
    """Per-core sorted-update metadata + SPMD-uniform chunk->tile windows."""
    per_core = []
    for b in range(B):
        t_flat = np.asarray(indexes[b], dtype=np.int64).reshape(-1)  # (n, k) order
        order = np.argsort(t_flat, kind="stable")
        t_sorted = t_flat[order]
        src = (order // K).astype(np.int16)
        counts = np.bincount(t_flat, minlength=S)
        r = (1.0 / np.maximum(counts, 1)).astype(np.float32)
        per_core.append((t_sorted, src, r))

    # Union coverage: chunk c touches output tiles [lo[c], hi[c]] across cores.
    lo = np.full(N_CHUNKS, N_TILES, np.int64)
    hi = np.full(N_CHUNKS, -1, np.int64)
    for t_sorted, _, _ in per_core:
        tc_lo = t_sorted.reshape(N_CHUNKS, P)[:, 0] // P
        tc_hi = t_sorted.reshape(N_CHUNKS, P)[:, -1] // P
        lo = np.minimum(lo, tc_lo)
        hi = np.maximum(hi, tc_hi)
    lo = lo.astype(int)
    hi = hi.astype(int)

    src_cols = np.zeros((B, P, (N * K) // 16), np.int16)  # wrapped idx layout
    shift_cols = np.zeros((B, P, N_CHUNKS), np.float32)
    r_cols = np.zeros((B, P, N_TILES), np.float32)
    for b in range(B):
        t_sorted, src, r = per_core[b]
        # dma_gather wrapped index layout: idx j of gather g lives at
        # [j % 16, j // 16] within each 16-partition group, replicated x8.
        w = src.reshape(N_CHUNKS * P // 16, 16).T  # [16, 128]
        src_cols[b] = np.tile(w, (8, 1))
        r_cols[b] = r.reshape(N_TILES, P).T
        ts_chunks = t_sorted.reshape(N_CHUNKS, P)
        for c in range(N_CHUNKS):
            shift_cols[b, :, c] = (ts_chunks[c] - P * lo[c]).astype(np.float32)
    return lo, hi, src_cols, shift_cols, r_cols


def _build_kernel(lo, hi):
    f32 = mybir.dt.float32
    f32r = mybir.dt.float32r
    spans = [hi[c] - lo[c] + 1 for c in range(N_CHUNKS)]
    wmax = P * max(spans)
    # per tile: contributing chunks (ordered) for PSUM start/stop flags
    tile_chunks = {tau: [c for c in range(N_CHUNKS) if lo[c] <= tau <= hi[c]]
                   for tau in range(N_TILES)}

    nc = bacc.Bacc("TRN2", target_bir_lowering=False, debug=False)
    seq = nc.dram_tensor("seq", [S, DSEQ], f32, kind="ExternalInput")
    g = nc.dram_tensor("g", [N, DG], f32, kind="ExternalInput")
    srcm = nc.dram_tensor("srcm", [P, (N * K) // 16], mybir.dt.int16,
                          kind="ExternalInput")
    shiftm = nc.dram_tensor("shiftm", [P, N_CHUNKS], f32, kind="ExternalInput")
    rm = nc.dram_tensor("rm", [P, N_TILES], f32, kind="ExternalInput")
    iotam = nc.dram_tensor("iotam", [P, wmax], f32, kind="ExternalInput")
    enc = nc.dram_tensor("enc", [S, DG + DSEQ], f32, kind="ExternalOutput")

    idx_cols = (P * CPG) // 16  # 32 index columns per gather

    with tile.TileContext(nc) as tc:
        with (
            tc.tile_pool(name="const", bufs=1) as const,
            tc.tile_pool(name="gath", bufs=N_GATHERS) as gather_pool,
            tc.tile_pool(name="hilo", bufs=8) as hilo_pool,
            tc.tile_pool(name="eq", bufs=N_CHUNKS) as eq_pool,
            tc.tile_pool(name="out", bufs=4) as out_pool,
            tc.tile_pool(name="psum", bufs=8, space="PSUM") as psum_pool,
        ):
            # --- Sync queue: metadata loads (gathers depend only on src_sb).
            src_sb = const.tile([P, (N * K) // 16], mybir.dt.int16)
            nc.sync.dma_start(out=src_sb[:], in_=srcm[:, :])
            shift_sb = const.tile([P, N_CHUNKS], f32)
            nc.sync.dma_start(out=shift_sb[:], in_=shiftm[:, :])
            r_sb = const.tile([P, N_TILES], f32)
            nc.sync.dma_start(out=r_sb[:], in_=rm[:, :])
            iota_f = const.tile([P, wmax], f32)
            nc.sync.dma_start(out=iota_f[:], in_=iotam[:, :])

            # --- Scalar(ACT) HWDGE ring: seq rows into the out super-tiles
            # right away (no dependencies; distinct ring from the out stores).
            ots = []
            for st in range(N_TILES // TPS):
                ot = out_pool.tile([P, TPS, DG + DSEQ], f32)
                nc.scalar.dma_start(
                    out=ot[:, :, DG:],
                    in_=seq[:, :].rearrange("(t p) d -> p t d", p=P)[
                        :, st * TPS:(st + 1) * TPS, :],
                )
                ots.append(ot)

            # --- GpSimd: the 4 row gathers, back to back.
            gathers = []
            for gi in range(N_GATHERS):
                gt = gather_pool.tile([P, CPG, DG], f32)
                nc.gpsimd.dma_gather(
                    gt[:], g[:, :],
                    src_sb[:, gi * idx_cols:(gi + 1) * idx_cols],
                    P * CPG, P * CPG, DG, elem_step=DG,
                )
                gathers.append(gt)

            # epsr on DVE (needed late, by epilogues only)
            epsr_sb = const.tile([P, N_TILES], f32)
            nc.vector.tensor_scalar(
                out=epsr_sb[:], in0=r_sb[:], scalar1=EPS, scalar2=None,
                op0=mybir.AluOpType.mult,
            )

            # --- Chunk-major pipeline, emission order == execution order.
            psums = {}
            seen = {tau: 0 for tau in range(N_TILES)}
            done_tiles_in_st = [0] * (N_TILES // TPS)

            def finish_tile(tau):
                """Epilogue on ACT + super-tile store when complete."""
                st, ti = divmod(tau, TPS)
                oslice = ots[st][:, ti, :DG]
                if tau in psums:
                    nc.scalar.activation(
                        out=oslice, in_=psums[tau][:],
                        func=mybir.ActivationFunctionType.Identity,
                        bias=epsr_sb[:, tau:tau + 1],
                        scale=r_sb[:, tau:tau + 1],
                    )
                else:
                    nc.vector.memset(oslice, EPS)
                done_tiles_in_st[st] += 1
                if done_tiles_in_st[st] == TPS:
                    nc.sync.dma_start(
                        out=enc[st * TPS * P:(st + 1) * TPS * P, :].rearrange(
                            "(t p) d -> p t d", p=P),
                        in_=ots[st][:],
                    )

            for c in range(N_CHUNKS):
                gi, ci = c // CPG, c % CPG
                raw = gathers[gi][:, ci, :]
                # selection window (DVE; ready early, fills DVE while gathers run)
                eqt = eq_pool.tile([P, P * spans[c]], f32r, tag="eq")
                nc.vector.tensor_scalar(
                    out=eqt[:], in0=iota_f[:, :P * spans[c]],
                    scalar1=shift_sb[:, c:c + 1], scalar2=None,
                    op0=mybir.AluOpType.is_equal,
                )
                # f32r hi/lo split of this chunk's rows
                ghi = hilo_pool.tile([P, DG], f32r, tag="ghi")
                nc.scalar.copy(out=ghi[:], in_=raw)
                glo = hilo_pool.tile([P, DG], f32r, tag="glo")
                nc.vector.tensor_tensor(
                    out=glo[:], in0=raw, in1=ghi[:].bitcast(f32),
                    op=mybir.AluOpType.subtract,
                )
                # matmuls for every tile this chunk's window covers
                for tau in range(lo[c], hi[c] + 1):
                    clist = tile_chunks[tau]
                    if tau not in psums:
                        psums[tau] = psum_pool.tile([P, DG], f32, space="PSUM",
                                                    name=f"ps{tau}", tag="ps")
                    first = clist[0] == c
                    last = clist[-1] == c
                    off = P * (tau - lo[c])
                    nc.tensor.matmul(
                        out=psums[tau][:], lhsT=eqt[:, off:off + P], rhs=ghi[:],
                        start=first, stop=False,
                    )
                    nc.tensor.matmul(
                        out=psums[tau][:], lhsT=eqt[:, off:off + P], rhs=glo[:],
                        start=False, stop=last,
                    )
                    seen[tau] += 1
                    if seen[tau] == len(clist):
                        finish_tile(tau)
            # tiles covered by no chunk window (possible for adversarial index
            # distributions): plain eps fill
            for tau in range(N_TILES):
                if not tile_chunks[tau]:
                    finish_tile(tau)
    nc.compile()
    return nc


def kernel(seq_output, graph_output, hidden, indexes, _trace=False):
    global LAST_EXEC_NS, LAST_RESULTS
    seq_output = np.ascontiguousarray(np.asarray(seq_output, dtype=np.float32))
    graph_output = np.ascontiguousarray(np.asarray(graph_output, dtype=np.float32))
    hidden_np = np.asarray(hidden)

    lo, hi, src_cols, shift_cols, r_cols = _host_metadata(indexes)
    nc = _build_kernel(lo, hi)

    wmax = P * max(hi[c] - lo[c] + 1 for c in range(N_CHUNKS))
    iota_full = np.broadcast_to(
        np.arange(wmax, dtype=np.float32), (P, wmax)
    ).copy()

    in_maps = [
        {
            "seq": seq_output[b],
            "g": graph_output[b],
            "srcm": np.ascontiguousarray(src_cols[b]),
            "shiftm": np.ascontiguousarray(shift_cols[b]),
            "rm": np.ascontiguousarray(r_cols[b]),
            "iotam": iota_full,
        }
        for b in range(B)
    ]
    res = run_bass_kernel_spmd(nc, in_maps, core_ids=list(range(B)), trace=_trace)
    LAST_EXEC_NS = res.exec_time_ns
    LAST_RESULTS = res
    enc = np.stack([res.results[b]["enc"] for b in range(B)], axis=0)
    hidden_flat = np.ascontiguousarray(hidden_np.reshape(hidden_np.shape[0], -1))
    return enc, hidden_flat


# revision 12
# speedup vs baseline: 1.3656x; 1.1535x over previous
"""Trainium2 Bass kernel for nn_DecoderTransformer segment_reduce problem.

Computes, per batch sample b (one NeuronCore each, 8 cores total):
    sums[s, :]   = sum over (n, k) with indexes[b, n, k] == s of graph_output[b, n, :]
    counts[s]    = multiplicity of s in indexes[b]
    graph_hidden = (sums + 1e-8) / max(counts, 1)
    enc[b]       = concat([graph_hidden, seq_output[b]], axis=-1)   # [2048, 1024]
Returns (enc [8, 2048, 1024] f32, hidden [8, 1024] f32 passthrough).

Device algorithm (per core):
  Host sorts the 2048 (n, k) updates by target s; the sorted stream is cut in
  16 chunks of 128. The source rows of graph_output are fetched in sorted
  order by 4 dma_gather ops (512 rows each; rows land on partitions in chunk
  layout [128, 4, 512]). Per chunk the rows are split into an exact float32r
  hi/lo pair (ACT copy + DVE subtract), and one is_equal tensor_scalar
  against a host-provided iota row builds the selection matrix for the window
  of output tiles the chunk's targets span (window union across cores keeps
  the program SPMD-uniform). The scatter-add is Sel.T @ rows on the tensor
  engine as two float32r matmuls, accumulated in PSUM across chunks. The
  PSUM->SBUF pass (ACT) fuses (sums + eps) * r with r = 1/max(counts, 1)
  host-precomputed from the index metadata, writing into [128, 4096]
  super-tiles interleaved with seq_output so output stores are 4 contiguous
  2 MiB DMAs. Instructions are emitted chunk-major in execution order: the
  per-engine queues are strict FIFO, so emission order is schedule order.
"""

import numpy as np

import concourse.bass as bass
import concourse.bacc as bacc
import concourse.tile as tile
from concourse import mybir
from concourse.bass_utils import run_bass_kernel_spmd

B, S, N, K = 8, 2048, 512, 4
DG, DSEQ, H = 512, 512, 1024
P = 128
N_CHUNKS = (N * K) // P  # 16
N_TILES = S // P  # 16
N_GATHERS = 4  # 2048 rows in 4 dma_gather ops (>1024 idxs per op crashes Q7)
CPG = N_CHUNKS // N_GATHERS  # chunks per gather
TPS = 4  # tiles per output super-tile
EPS = 1e-8

# Filled by kernel() on every call; read by test harnesses.
LAST_EXEC_NS = None
LAST_RESULTS = None


def _host_metadata(indexes):
    """Per-core sorted-update metadata + SPMD-uniform chunk->tile windows."""
    per_core = []
    for b in range(B):
        t_flat = np.asarray(indexes[b], dtype=np.int64).reshape(-1)  # (n, k) order
        order = np.argsort(t_flat, kind="stable")
        t_sorted = t_flat[order]
        src = (order // K).astype(np.int32)
        counts = np.bincount(t_flat, minlength=S)
        r = (1.0 / np.maximum(counts, 1)).astype(np.float32)
        per_core.append((t_sorted, src, r))

    # Union coverage: chunk c touches output tiles [lo[c], hi[c]] across cores.
    lo = np.full(N_CHUNKS, N_TILES, np.int64)
    hi = np.full(N_CHUNKS, -1, np.int64)
    for t_sorted, _, _ in per_core:
        tc_lo = t_sorted.reshape(N_CHUNKS, P)[:, 0] // P
        tc_hi = t_sorted.reshape(N_CHUNKS, P)[:, -1] // P
        lo = np.minimum(lo, tc_lo)
        hi = np.maximum(hi, tc_hi)
    lo = lo.astype(int)
    hi = hi.astype(int)

    src_cols = np.zeros((B, P, N_CHUNKS), np.int32)  # [p, c] -> source row
    shift_cols = np.zeros((B, P, N_CHUNKS), np.float32)
    r_cols = np.zeros((B, P, N_TILES), np.float32)
    for b in range(B):
        t_sorted, src, r = per_core[b]
        src_cols[b] = src.reshape(N_CHUNKS, P).T.astype(np.int32)
        r_cols[b] = r.reshape(N_TILES, P).T
        ts_chunks = t_sorted.reshape(N_CHUNKS, P)
        for c in range(N_CHUNKS):
            shift_cols[b, :, c] = (ts_chunks[c] - P * lo[c]).astype(np.float32)
    return lo, hi, src_cols, shift_cols, r_cols


def _build_kernel(lo, hi):
    f32 = mybir.dt.float32
    f32r = mybir.dt.float32r
    spans = [hi[c] - lo[c] + 1 for c in range(N_CHUNKS)]
    wmax = P * max(spans)
    # per tile: contributing chunks (ordered) for PSUM start/stop flags
    tile_chunks = {tau: [c for c in range(N_CHUNKS) if lo[c] <= tau <= hi[c]]
                   for tau in range(N_TILES)}

    nc = bacc.Bacc("TRN2", target_bir_lowering=False, debug=False)
    seq = nc.dram_tensor("seq", [S, DSEQ], f32, kind="ExternalInput")
    g = nc.dram_tensor("g", [N, DG], f32, kind="ExternalInput")
    srcm = nc.dram_tensor("srcm", [P, N_CHUNKS], mybir.dt.int32,
                          kind="ExternalInput")
    shiftm = nc.dram_tensor("shiftm", [P, N_CHUNKS], f32, kind="ExternalInput")
    rm = nc.dram_tensor("rm", [P, N_TILES], f32, kind="ExternalInput")
    iotam = nc.dram_tensor("iotam", [P, wmax], f32, kind="ExternalInput")
    enc = nc.dram_tensor("enc", [S, DG + DSEQ], f32, kind="ExternalOutput")


    with tile.TileContext(nc) as tc:
        with (
            tc.tile_pool(name="const", bufs=1) as const,
            tc.tile_pool(name="gath", bufs=N_CHUNKS) as gather_pool,
            tc.tile_pool(name="hilo", bufs=8) as hilo_pool,
            tc.tile_pool(name="eq", bufs=N_CHUNKS) as eq_pool,
            tc.tile_pool(name="out", bufs=4) as out_pool,
            tc.tile_pool(name="psum", bufs=8, space="PSUM") as psum_pool,
        ):
            # --- Sync queue: metadata loads (gathers depend only on src_sb).
            src_sb = const.tile([P, N_CHUNKS], mybir.dt.int32)
            nc.sync.dma_start(out=src_sb[:], in_=srcm[:, :])
            shift_sb = const.tile([P, N_CHUNKS], f32)
            nc.sync.dma_start(out=shift_sb[:], in_=shiftm[:, :])
            r_sb = const.tile([P, N_TILES], f32)
            nc.sync.dma_start(out=r_sb[:], in_=rm[:, :])
            iota_f = const.tile([P, wmax], f32)
            nc.sync.dma_start(out=iota_f[:], in_=iotam[:, :])

            # --- Scalar(ACT) HWDGE ring: seq rows into the out super-tiles
            # right away (no dependencies; distinct ring from the out stores).
            ots = []
            for st in range(N_TILES // TPS):
                ot = out_pool.tile([P, TPS, DG + DSEQ], f32)
                nc.scalar.dma_start(
                    out=ot[:, :, DG:],
                    in_=seq[:, :].rearrange("(t p) d -> p t d", p=P)[
                        :, st * TPS:(st + 1) * TPS, :],
                )
                ots.append(ot)

            # --- GpSimd: 16 per-chunk row gathers (single-offset indirect
            # DMA: out[p, :] = g[src_sb[p, c], :]; plain SWDGE, no ucode
            # library load on the critical path). Multi-offset APs and the
            # dma_gather ucode both fail on HW (wrong results / 13.5 us
            # library-load stall), so one op per 128-row chunk it is.
            gathers = []
            for c in range(N_CHUNKS):
                gt = gather_pool.tile([P, DG], f32, name=f"gt{c}", tag="gt")
                nc.gpsimd.indirect_dma_start(
                    out=gt[:], out_offset=None, in_=g[:, :],
                    in_offset=bass.IndirectOffsetOnAxis(
                        ap=src_sb[:, c:c + 1], axis=0),
                )
                gathers.append(gt)

            # epsr on DVE (needed late, by epilogues only)
            epsr_sb = const.tile([P, N_TILES], f32)
            nc.vector.tensor_scalar(
                out=epsr_sb[:], in0=r_sb[:], scalar1=EPS, scalar2=None,
                op0=mybir.AluOpType.mult,
            )

            # --- Chunk-major pipeline, emission order == execution order.
            psums = {}
            seen = {tau: 0 for tau in range(N_TILES)}
            done_tiles_in_st = [0] * (N_TILES // TPS)

            def finish_tile(tau):
                """Epilogue on ACT + super-tile store when complete."""
                st, ti = divmod(tau, TPS)
                oslice = ots[st][:, ti, :DG]
                if tau in psums:
                    nc.scalar.activation(
                        out=oslice, in_=psums[tau][:],
                        func=mybir.ActivationFunctionType.Identity,
                        bias=epsr_sb[:, tau:tau + 1],
                        scale=r_sb[:, tau:tau + 1],
                    )
                else:
                    nc.vector.memset(oslice, EPS)
                done_tiles_in_st[st] += 1
                if done_tiles_in_st[st] == TPS:
                    nc.sync.dma_start(
                        out=enc[st * TPS * P:(st + 1) * TPS * P, :].rearrange(
                            "(t p) d -> p t d", p=P),
                        in_=ots[st][:],
                    )

            for c in range(N_CHUNKS):
                raw = gathers[c][:]
                # selection windows (DVE; ready early, fill DVE while
                # gathers run): f32r flavor for the hi pass, bf16 for lo
                eqt = eq_pool.tile([P, P * spans[c]], f32r, tag="eq")
                nc.vector.tensor_scalar(
                    out=eqt[:], in0=iota_f[:, :P * spans[c]],
                    scalar1=shift_sb[:, c:c + 1], scalar2=None,
                    op0=mybir.AluOpType.is_equal,
                )
                eqb = eq_pool.tile([P, P * spans[c]], mybir.dt.bfloat16,
                                   tag="eqb")
                nc.vector.tensor_scalar(
                    out=eqb[:], in0=iota_f[:, :P * spans[c]],
                    scalar1=shift_sb[:, c:c + 1], scalar2=None,
                    op0=mybir.AluOpType.is_equal,
                )
                # f32r hi/lo split of this chunk's rows
                ghi = hilo_pool.tile([P, DG], f32r, tag="ghi")
                nc.scalar.copy(out=ghi[:], in_=raw)
                # lo residual in bf16: |lo| <= 2^-12 |G|, bf16 keeps 8 more
                # bits -> combined ~2^-21 relative error, PE cost 213 ns/row
                glo = hilo_pool.tile([P, DG], mybir.dt.bfloat16, tag="glo")
                nc.vector.tensor_tensor(
                    out=glo[:], in0=raw, in1=ghi[:].bitcast(f32),
                    op=mybir.AluOpType.subtract,
                )
                # matmuls for every tile this chunk's window covers
                for tau in range(lo[c], hi[c] + 1):
                    clist = tile_chunks[tau]
                    if tau not in psums:
                        psums[tau] = psum_pool.tile([P, DG], f32, space="PSUM",
                                                    name=f"ps{tau}", tag="ps")
                    first = clist[0] == c
                    last = clist[-1] == c
                    off = P * (tau - lo[c])
                    nc.tensor.matmul(
                        out=psums[tau][:], lhsT=eqt[:, off:off + P], rhs=ghi[:],
                        start=first, stop=False,
                    )
                    nc.tensor.matmul(
                        out=psums[tau][:], lhsT=eqb[:, off:off + P], rhs=glo[:],
                        start=False, stop=last,
                    )
                    seen[tau] += 1
                    if seen[tau] == len(clist):
                        finish_tile(tau)
            # tiles covered by no chunk window (possible for adversarial index
            # distributions): plain eps fill
            for tau in range(N_TILES):
                if not tile_chunks[tau]:
                    finish_tile(tau)
    nc.compile()
    return nc


def kernel(seq_output, graph_output, hidden, indexes, _trace=False):
    global LAST_EXEC_NS, LAST_RESULTS
    seq_output = np.ascontiguousarray(np.asarray(seq_output, dtype=np.float32))
    graph_output = np.ascontiguousarray(np.asarray(graph_output, dtype=np.float32))
    hidden_np = np.asarray(hidden)

    lo, hi, src_cols, shift_cols, r_cols = _host_metadata(indexes)
    nc = _build_kernel(lo, hi)

    wmax = P * max(hi[c] - lo[c] + 1 for c in range(N_CHUNKS))
    iota_full = np.broadcast_to(
        np.arange(wmax, dtype=np.float32), (P, wmax)
    ).copy()

    in_maps = [
        {
            "seq": seq_output[b],
            "g": graph_output[b],
            "srcm": np.ascontiguousarray(src_cols[b]),
            "shiftm": np.ascontiguousarray(shift_cols[b]),
            "rm": np.ascontiguousarray(r_cols[b]),
            "iotam": iota_full,
        }
        for b in range(B)
    ]
    res = run_bass_kernel_spmd(nc, in_maps, core_ids=list(range(B)), trace=_trace)
    LAST_EXEC_NS = res.exec_time_ns
    LAST_RESULTS = res
    enc = np.stack([res.results[b]["enc"] for b in range(B)], axis=0)
    hidden_flat = np.ascontiguousarray(hidden_np.reshape(hidden_np.shape[0], -1))
    return enc, hidden_flat


# revision 13
# speedup vs baseline: 1.4734x; 1.0789x over previous
"""Trainium2 Bass kernel for nn_DecoderTransformer segment_reduce problem.

Computes, per batch sample b (one NeuronCore each, 8 cores total):
    sums[s, :]   = sum over (n, k) with indexes[b, n, k] == s of graph_output[b, n, :]
    counts[s]    = multiplicity of s in indexes[b]
    graph_hidden = (sums + 1e-8) / max(counts, 1)
    enc[b]       = concat([graph_hidden, seq_output[b]], axis=-1)   # [2048, 1024]
Returns (enc [8, 2048, 1024] f32, hidden [8, 1024] f32 passthrough).

Device algorithm (per core):
  Host sorts the 2048 (n, k) updates by target s; the sorted stream is cut in
  16 chunks of 128. The source rows of graph_output are fetched in sorted
  order by 4 dma_gather ops (512 rows each; rows land on partitions in chunk
  layout [128, 4, 512]). Per chunk the rows are split into an exact float32r
  hi/lo pair (ACT copy + DVE subtract), and one is_equal tensor_scalar
  against a host-provided iota row builds the selection matrix for the window
  of output tiles the chunk's targets span (window union across cores keeps
  the program SPMD-uniform). The scatter-add is Sel.T @ rows on the tensor
  engine as two float32r matmuls, accumulated in PSUM across chunks. The
  PSUM->SBUF pass (ACT) fuses (sums + eps) * r with r = 1/max(counts, 1)
  host-precomputed from the index metadata, writing into [128, 4096]
  super-tiles interleaved with seq_output so output stores are 4 contiguous
  2 MiB DMAs. Instructions are emitted chunk-major in execution order: the
  per-engine queues are strict FIFO, so emission order is schedule order.
"""

import numpy as np

import concourse.bass as bass
import concourse.bacc as bacc
import concourse.tile as tile
from concourse import mybir
from concourse.bass_utils import run_bass_kernel_spmd

B, S, N, K = 8, 2048, 512, 4
DG, DSEQ, H = 512, 512, 1024
P = 128
N_CHUNKS = (N * K) // P  # 16
N_TILES = S // P  # 16
N_GATHERS = 4  # 2048 rows in 4 dma_gather ops (>1024 idxs per op crashes Q7)
CPG = N_CHUNKS // N_GATHERS  # chunks per gather
TPS = 4  # tiles per output super-tile
EPS = 1e-8

# Filled by kernel() on every call; read by test harnesses.
LAST_EXEC_NS = None
LAST_RESULTS = None


def _host_metadata(indexes):
    """Per-core sorted-update metadata + SPMD-uniform chunk->tile windows."""
    per_core = []
    for b in range(B):
        t_flat = np.asarray(indexes[b], dtype=np.int64).reshape(-1)  # (n, k) order
        order = np.argsort(t_flat, kind="stable")
        t_sorted = t_flat[order]
        src = (order // K).astype(np.int32)
        counts = np.bincount(t_flat, minlength=S)
        r = (1.0 / np.maximum(counts, 1)).astype(np.float32)
        per_core.append((t_sorted, src, r))

    # Union coverage: chunk c touches output tiles [lo[c], hi[c]] across cores.
    lo = np.full(N_CHUNKS, N_TILES, np.int64)
    hi = np.full(N_CHUNKS, -1, np.int64)
    for t_sorted, _, _ in per_core:
        tc_lo = t_sorted.reshape(N_CHUNKS, P)[:, 0] // P
        tc_hi = t_sorted.reshape(N_CHUNKS, P)[:, -1] // P
        lo = np.minimum(lo, tc_lo)
        hi = np.maximum(hi, tc_hi)
    lo = lo.astype(int)
    hi = hi.astype(int)

    src_cols = np.zeros((B, P, N_CHUNKS), np.int32)  # [p, c] -> source row
    shift_cols = np.zeros((B, P, N_CHUNKS), np.float32)
    r_cols = np.zeros((B, P, N_TILES), np.float32)
    for b in range(B):
        t_sorted, src, r = per_core[b]
        src_cols[b] = src.reshape(N_CHUNKS, P).T.astype(np.int32)
        r_cols[b] = r.reshape(N_TILES, P).T
        ts_chunks = t_sorted.reshape(N_CHUNKS, P)
        for c in range(N_CHUNKS):
            shift_cols[b, :, c] = (ts_chunks[c] - P * lo[c]).astype(np.float32)
    return lo, hi, src_cols, shift_cols, r_cols


def _build_kernel(lo, hi):
    f32 = mybir.dt.float32
    f32r = mybir.dt.float32r
    spans = [hi[c] - lo[c] + 1 for c in range(N_CHUNKS)]
    wmax = P * max(spans)
    # per tile: contributing chunks (ordered) for PSUM start/stop flags
    tile_chunks = {tau: [c for c in range(N_CHUNKS) if lo[c] <= tau <= hi[c]]
                   for tau in range(N_TILES)}

    nc = bacc.Bacc("TRN2", target_bir_lowering=False, debug=False)
    seq = nc.dram_tensor("seq", [S, DSEQ], f32, kind="ExternalInput")
    g = nc.dram_tensor("g", [N, DG], f32, kind="ExternalInput")
    srcm = nc.dram_tensor("srcm", [P, N_CHUNKS], mybir.dt.int32,
                          kind="ExternalInput")
    shiftm = nc.dram_tensor("shiftm", [P, N_CHUNKS], f32, kind="ExternalInput")
    rm = nc.dram_tensor("rm", [P, N_TILES], f32, kind="ExternalInput")
    iotam = nc.dram_tensor("iotam", [P, wmax], f32, kind="ExternalInput")
    enc = nc.dram_tensor("enc", [S, DG + DSEQ], f32, kind="ExternalOutput")


    with tile.TileContext(nc) as tc:
        with (
            tc.tile_pool(name="const", bufs=1) as const,
            tc.tile_pool(name="gath", bufs=N_CHUNKS) as gather_pool,
            tc.tile_pool(name="hilo", bufs=8) as hilo_pool,
            tc.tile_pool(name="eq", bufs=N_CHUNKS) as eq_pool,
            tc.tile_pool(name="out", bufs=4) as out_pool,
            tc.tile_pool(name="psum", bufs=8, space="PSUM") as psum_pool,
        ):
            # --- Sync queue: metadata loads (gathers depend only on src_sb).
            src_sb = const.tile([P, N_CHUNKS], mybir.dt.int32)
            nc.sync.dma_start(out=src_sb[:], in_=srcm[:, :])
            shift_sb = const.tile([P, N_CHUNKS], f32)
            nc.sync.dma_start(out=shift_sb[:], in_=shiftm[:, :])
            r_sb = const.tile([P, N_TILES], f32)
            nc.sync.dma_start(out=r_sb[:], in_=rm[:, :])
            iota_f = const.tile([P, wmax], f32)
            nc.sync.dma_start(out=iota_f[:], in_=iotam[:, :])

            # --- Scalar(ACT) HWDGE ring: seq rows into the out super-tiles
            # right away (no dependencies; distinct ring from the out stores).
            ots = []
            for st in range(N_TILES // TPS):
                ot = out_pool.tile([P, TPS, DG + DSEQ], f32)
                nc.scalar.dma_start(
                    out=ot[:, :, DG:],
                    in_=seq[:, :].rearrange("(t p) d -> p t d", p=P)[
                        :, st * TPS:(st + 1) * TPS, :],
                )
                ots.append(ot)

            # --- GpSimd: 16 per-chunk row gathers (single-offset indirect
            # DMA: out[p, :] = g[src_sb[p, c], :]; plain SWDGE, no ucode
            # library load on the critical path). Multi-offset APs and the
            # dma_gather ucode both fail on HW (wrong results / 13.5 us
            # library-load stall), so one op per 128-row chunk it is.
            gathers = []
            for c in range(N_CHUNKS):
                gt = gather_pool.tile([P, DG], f32, name=f"gt{c}", tag="gt")
                nc.gpsimd.indirect_dma_start(
                    out=gt[:], out_offset=None, in_=g[:, :],
                    in_offset=bass.IndirectOffsetOnAxis(
                        ap=src_sb[:, c:c + 1], axis=0),
                )
                gathers.append(gt)

            # epsr on DVE (needed late, by epilogues only)
            epsr_sb = const.tile([P, N_TILES], f32)
            nc.vector.tensor_scalar(
                out=epsr_sb[:], in0=r_sb[:], scalar1=EPS, scalar2=None,
                op0=mybir.AluOpType.mult,
            )

            # --- Chunk-major pipeline, emission order == execution order.
            psums = {}
            seen = {tau: 0 for tau in range(N_TILES)}
            done_tiles_in_st = [0] * (N_TILES // TPS)

            def finish_tile(tau):
                """Epilogue on ACT + super-tile store when complete."""
                st, ti = divmod(tau, TPS)
                oslice = ots[st][:, ti, :DG]
                if tau in psums:
                    nc.scalar.activation(
                        out=oslice, in_=psums[tau][:],
                        func=mybir.ActivationFunctionType.Identity,
                        bias=epsr_sb[:, tau:tau + 1],
                        scale=r_sb[:, tau:tau + 1],
                    )
                else:
                    nc.vector.memset(oslice, EPS)
                # per-tile store: 512 KiB contiguous rows, issued as soon as
                # this tile's epilogue lands (short tail after the last chunk)
                nc.sync.dma_start(
                    out=enc[tau * P:(tau + 1) * P, :],
                    in_=ots[st][:, ti, :],
                )

            for c in range(N_CHUNKS):
                raw = gathers[c][:]
                # selection windows (DVE; ready early, fill DVE while
                # gathers run): f32r flavor for the hi pass, bf16 for lo
                eqt = eq_pool.tile([P, P * spans[c]], f32r, tag="eq")
                nc.vector.tensor_scalar(
                    out=eqt[:], in0=iota_f[:, :P * spans[c]],
                    scalar1=shift_sb[:, c:c + 1], scalar2=None,
                    op0=mybir.AluOpType.is_equal,
                )
                eqb = eq_pool.tile([P, P * spans[c]], mybir.dt.bfloat16,
                                   tag="eqb")
                nc.vector.tensor_scalar(
                    out=eqb[:], in0=iota_f[:, :P * spans[c]],
                    scalar1=shift_sb[:, c:c + 1], scalar2=None,
                    op0=mybir.AluOpType.is_equal,
                )
                # f32r hi/lo split of this chunk's rows
                ghi = hilo_pool.tile([P, DG], f32r, tag="ghi")
                nc.scalar.copy(out=ghi[:], in_=raw)
                # lo residual in bf16: |lo| <= 2^-12 |G|, bf16 keeps 8 more
                # bits -> combined ~2^-21 relative error, PE cost 213 ns/row
                glo = hilo_pool.tile([P, DG], mybir.dt.bfloat16, tag="glo")
                nc.vector.tensor_tensor(
                    out=glo[:], in0=raw, in1=ghi[:].bitcast(f32),
                    op=mybir.AluOpType.subtract,
                )
                # matmuls for every tile this chunk's window covers
                for tau in range(lo[c], hi[c] + 1):
                    clist = tile_chunks[tau]
                    if tau not in psums:
                        psums[tau] = psum_pool.tile([P, DG], f32, space="PSUM",
                                                    name=f"ps{tau}", tag="ps")
                    first = clist[0] == c
                    last = clist[-1] == c
                    off = P * (tau - lo[c])
                    nc.tensor.matmul(
                        out=psums[tau][:], lhsT=eqt[:, off:off + P], rhs=ghi[:],
                        start=first, stop=False,
                    )
                    nc.tensor.matmul(
                        out=psums[tau][:], lhsT=eqb[:, off:off + P], rhs=glo[:],
                        start=False, stop=last,
                    )
                    seen[tau] += 1
                    if seen[tau] == len(clist):
                        finish_tile(tau)
            # tiles covered by no chunk window (possible for adversarial index
            # distributions): plain eps fill
            for tau in range(N_TILES):
                if not tile_chunks[tau]:
                    finish_tile(tau)
    nc.compile()
    return nc


def kernel(seq_output, graph_output, hidden, indexes, _trace=False):
    global LAST_EXEC_NS, LAST_RESULTS
    seq_output = np.ascontiguousarray(np.asarray(seq_output, dtype=np.float32))
    graph_output = np.ascontiguousarray(np.asarray(graph_output, dtype=np.float32))
    hidden_np = np.asarray(hidden)

    lo, hi, src_cols, shift_cols, r_cols = _host_metadata(indexes)
    nc = _build_kernel(lo, hi)

    wmax = P * max(hi[c] - lo[c] + 1 for c in range(N_CHUNKS))
    iota_full = np.broadcast_to(
        np.arange(wmax, dtype=np.float32), (P, wmax)
    ).copy()

    in_maps = [
        {
            "seq": seq_output[b],
            "g": graph_output[b],
            "srcm": np.ascontiguousarray(src_cols[b]),
            "shiftm": np.ascontiguousarray(shift_cols[b]),
            "rm": np.ascontiguousarray(r_cols[b]),
            "iotam": iota_full,
        }
        for b in range(B)
    ]
    res = run_bass_kernel_spmd(nc, in_maps, core_ids=list(range(B)), trace=_trace)
    LAST_EXEC_NS = res.exec_time_ns
    LAST_RESULTS = res
    enc = np.stack([res.results[b]["enc"] for b in range(B)], axis=0)
    hidden_flat = np.ascontiguousarray(hidden_np.reshape(hidden_np.shape[0], -1))
    return enc, hidden_flat
